# revision 8
# baseline (speedup 1.0000x reference)
"""Gemma4 vision pooler (segment-reduce 4x4 average pooling) on 8 Trainium2 cores.

Strategy: data-parallel over the batch axis — one image per NeuronCore.
The reference's segment ids (from the regular 64x64 patch-position grid)
reduce to a 4x4 average pool over the grid, so the per-segment gather is
expressed as strided DMA access patterns and the reduction as a DVE
pairwise-add tree.  Memory-bound: each core streams its 18.9 MB image in,
writes 1.2 MB out.

Per-core pipeline (raw Bass, explicit semaphores; Tile is not used because
the staged walrus rejects multi-wait Drain instructions):
  - output tile t in {0,1} covers output rows t*128..t*128+127
    (partition p = (oy - 8t)*16 + ox for output o = oy*16 + ox)
  - 3 "chunk" gather-DMAs per tile: chunk dy = [128 part, (dx,e) 4608]
    for dy in {0,1,2}; 18 KB contiguous runs -> full DMA line rate
  - last grid row dy=3 is loaded as 4 separate [128, 1152] slices so the
    dependency tail after the final DMA is one add, not a whole tree
  - DVE: per chunk (dx0+dx1)+(dx2+dx3) tree, then z=(y0+y1)+y2, then
    r = (((z+s0)+s1)+s2)+s3 as slices land
  - ACT: scale by sqrt(H)/16 and issue the output store on its own
    HWDGE ring (overlaps SP's input ring)
"""

import numpy as np

S, H, O = 4096, 1152, 256
GRID, KERN = 64, 4
B_EXPECTED = 8
N_CORES = 8
SCALE = float(H) ** 0.5 / float(KERN * KERN)

_NC_CACHE: dict = {}


def build_nc(reps: int = 1):
    """Build the per-core Bass program. `reps` > 1 repeats the whole pipeline
    back-to-back in one NEFF (for on-device timing via wall-clock slope)."""
    from contextlib import ExitStack
    import concourse.bass as bass
    import concourse.mybir as mybir

    f32 = mybir.dt.float32
    nc = bass.Bass(trn_type="TRN2")
    h = nc.dram_tensor("h", [S, H], f32, kind="ExternalInput")
    out = nc.dram_tensor("out", [O, H], f32, kind="ExternalOutput")

    hap = h[:, :]
    # (oy, dy, ox, (dx e)): chunk source for (tile, dy) is v[t*8:(t+1)*8, dy]
    v = hap.rearrange("(oy dy ox dx) e -> oy dy ox (dx e)", oy=16, dy=4, ox=16, dx=4)
    # (oy, dy, ox, dx, e): slice source for (tile, j) is v2[t*8:(t+1)*8, 3, :, j]
    v2 = hap.rearrange("(oy dy ox dx) e -> oy dy ox dx e", oy=16, dy=4, ox=16, dx=4)

    with ExitStack() as ctx:
        sb = lambda name, free: ctx.enter_context(
            nc.sbuf_tensor(name, [128, free], f32)
        )
        c = [sb(f"c{dy}", 4608) for dy in range(3)]      # dy chunks, shared
        sl = [sb(f"sl{j}", 1152) for j in range(8)]      # dy=3 slices, per tile
        q0, q1 = sb("q0", 1152), sb("q1", 1152)
        y = [sb(f"y{dy}", 1152) for dy in range(3)]
        w, z = sb("w", 1152), sb("z", 1152)
        ra, rb = sb("ra", 1152), sb("rb", 1152)
        res = [sb(f"res{t}", 1152) for t in range(2)]

        # One semaphore per DMA-written buffer slot: completion order across
        # queued HWDGE DMAs is NOT guaranteed to be issue order (they fan out
        # over multiple queues), so "one shared sem >= 16*k" is unsound.
        sem_c = [ctx.enter_context(nc.semaphore(f"sem_c{dy}")) for dy in range(3)]
        sem_s = [ctx.enter_context(nc.semaphore(f"sem_s{j}")) for j in range(8)]
        sem_dve = ctx.enter_context(nc.semaphore("sem_dve"))
        sem_r = ctx.enter_context(nc.semaphore("sem_r"))
        sem_act = ctx.enter_context(nc.semaphore("sem_act"))
        sem_out = ctx.enter_context(nc.semaphore("sem_out"))

        # Semaphores retain values across NEFF executions on a core; a prior
        # run's leftovers make every wait_ge pass immediately (observed as
        # nondeterministic garbage).  Clear the whole kernel sem range, then
        # barrier via the NRT pseudo-barrier (its sems are outside the kernel
        # range, so it is safe while ours are still dirty).
        nc.gpsimd.dma_reset(nc._kernel_sem_range)
        nc.gpsimd.sem_clear(nc._kernel_sem_range)
        nc._nrt_pseudo_barrier()

        block = ctx.enter_context(nc.Block())

        @block.sync
        def _(sync):
            for i in range(reps):
                for t in range(2):
                    u = 2 * i + t  # global tile index
                    for dy in range(3):
                        if u > 0:  # chunk buffer reused; wait for y_dy of tile u-1
                            sync.wait_ge(sem_dve, 3 * (u - 1) + dy + 1)
                        sync.dma_start(
                            c[dy][:, :], v[t * 8 : (t + 1) * 8, dy]
                        ).then_inc(sem_c[dy], 16)
                    for j in range(4):
                        if i > 0:  # slice buffer reused; wait for tile (i-1, t) done
                            sync.wait_ge(sem_r, 2 * (i - 1) + t + 1)
                        sync.dma_start(
                            sl[4 * t + j][:, :], v2[t * 8 : (t + 1) * 8, 3, :, j]
                        ).then_inc(sem_s[4 * t + j], 16)
            sync.wait_ge(sem_out, 32 * reps)

        @block.vector
        def _(vector):
            add = mybir.AluOpType.add
            tt = vector.tensor_tensor
            for i in range(reps):
                for t in range(2):
                    u = 2 * i + t
                    for dy in range(3):
                        vector.wait_ge(sem_c[dy], 16 * (u + 1))
                        cc = c[dy][:, :].rearrange("p (dx e) -> p dx e", dx=4)
                        tt(q0[:, :], cc[:, 0], cc[:, 1], add)
                        tt(q1[:, :], cc[:, 2], cc[:, 3], add)
                        tt(y[dy][:, :], q0[:, :], q1[:, :], add).then_inc(sem_dve, 1)
                    tt(w[:, :], y[0][:, :], y[1][:, :], add)
                    tt(z[:, :], w[:, :], y[2][:, :], add)
                    vector.wait_ge(sem_s[4 * t + 0], 16 * (i + 1))
                    tt(ra[:, :], z[:, :], sl[4 * t + 0][:, :], add)
                    vector.wait_ge(sem_s[4 * t + 1], 16 * (i + 1))
                    if u > 0:  # rb still being read by ACT of tile u-1
                        vector.wait_ge(sem_act, u)
                    tt(rb[:, :], ra[:, :], sl[4 * t + 1][:, :], add)
                    vector.wait_ge(sem_s[4 * t + 2], 16 * (i + 1))
                    tt(ra[:, :], rb[:, :], sl[4 * t + 2][:, :], add)
                    vector.wait_ge(sem_s[4 * t + 3], 16 * (i + 1))
                    tt(rb[:, :], ra[:, :], sl[4 * t + 3][:, :], add).then_inc(sem_r, 1)

        @block.scalar
        def _(scalar):
            for i in range(reps):
                for t in range(2):
                    u = 2 * i + t
                    scalar.wait_ge(sem_r, u + 1)
                    if i > 0:  # res[t] still being read by out-DMA of (i-1, t)
                        scalar.wait_ge(sem_out, 32 * (i - 1) + 16 * (t + 1))
                    scalar.activation(
                        res[t][:, :],
                        rb[:, :],
                        mybir.ActivationFunctionType.Copy,
                        scale=SCALE,
                    ).then_inc(sem_act, 1)
                    # self-wait: ensure the activation's write has drained
                    # before this engine's DMA reads res[t]
                    scalar.wait_ge(sem_act, u + 1)
                    scalar.dma_start(
                        out[t * 128 : (t + 1) * 128, :], res[t][:, :]
                    ).then_inc(sem_out, 16)

    return nc


def build_nc_v2(reps: int = 1):
    """v2: partition = (oy%2, dy, ox), free = (dx, e).

    Each input DMA g in 0..7 loads grid rows oy=2g,2g+1 as ONE fully
    contiguous 2.36 MB DRAM span onto 128 partitions (measured 480 GB/s/core
    vs 194 for the v1 strided gather).  DVE pools dx in the free dim
    (2 adds/group), PE pools dy across partitions with a constant one-hot
    lhsT [128p -> 32 outputs] fp32 matmul (otherwise idle), ACT drains PSUM
    with the sqrt(H)/16 scale and stores 32 output rows per group.
    """
    from contextlib import ExitStack
    import concourse.bass as bass
    import concourse.mybir as mybir

    f32 = mybir.dt.float32
    nc = bass.Bass(trn_type="TRN2")
    h = nc.dram_tensor("h", [S, H], f32, kind="ExternalInput")
    pw = nc.dram_tensor("pw", [128, 32], f32, kind="ExternalInput")
    out = nc.dram_tensor("out", [O, H], f32, kind="ExternalOutput")

    # g-th slice: [oy2 2, dyox 64, (dx e) 4608] -> SBUF [128, 4608]
    vg = h[:, :].rearrange(
        "(g oy2 dyox dx) e -> g oy2 dyox (dx e)", g=8, oy2=2, dyox=64, dx=4
    )

    with ExitStack() as ctx:
        sb = lambda name, p, free: ctx.enter_context(
            nc.sbuf_tensor(name, [p, free], f32)
        )
        NS = 8                                               # in/xp ring depth
        tin = [sb(f"tin{k}", 128, 4608) for k in range(NS)]  # in slots
        q3 = sb("q3", 128, 2304)                             # dx pair sums
        xp = [sb(f"xp{k}", 128, 1152) for k in range(NS)]    # dx-pooled slots
        pwt = sb("pwt", 128, 32)                             # one-hot lhsT
        res = [sb(f"res{k}", 32, 1152) for k in range(4)]    # output staging
        psum = [
            ctx.enter_context(nc.psum_tensor(f"ps{k}", [32, 1152], f32))
            for k in range(2)
        ]

        sem_in = [ctx.enter_context(nc.semaphore(f"sem_in{k}")) for k in range(NS)]
        sem_w = ctx.enter_context(nc.semaphore("sem_w"))
        sem_a1 = ctx.enter_context(nc.semaphore("sem_a1"))
        sem_xp = ctx.enter_context(nc.semaphore("sem_xp"))
        sem_pe = ctx.enter_context(nc.semaphore("sem_pe"))
        sem_actc = ctx.enter_context(nc.semaphore("sem_actc"))
        sem_out = ctx.enter_context(nc.semaphore("sem_out"))

        nc.gpsimd.dma_reset(nc._kernel_sem_range)
        nc.gpsimd.sem_clear(nc._kernel_sem_range)
        nc._nrt_pseudo_barrier()

        block = ctx.enter_context(nc.Block())
        NG = 8 * reps  # total groups

        @block.sync
        def _(sync):
            sync.dma_start(pwt[:, :], pw[:, :]).then_inc(sem_w, 16)
            for G in range(NG):
                g = G % 8
                k = G % NS
                if G >= NS:  # slot reused; wait until add1 of G-NS consumed it
                    sync.wait_ge(sem_a1, G - NS + 1)
                sync.dma_start(tin[k][:, :], vg[g]).then_inc(sem_in[k], 16)
            sync.wait_ge(sem_out, 16 * NG)

        @block.vector
        def _(vector):
            add = mybir.AluOpType.add
            tt = vector.tensor_tensor
            for G in range(NG):
                k = G % NS
                vector.wait_ge(sem_in[k], 16 * (G // NS + 1))
                view = tin[k][:, :].rearrange("p (dx e) -> p dx e", dx=4)
                qv = q3[:, :].rearrange("p (j e) -> p j e", j=2)
                tt(qv, view[:, 0::2, :], view[:, 1::2, :], add).then_inc(sem_a1, 1)
                if G >= NS:  # xp slot reused; wait until PE of G-NS read it
                    vector.wait_ge(sem_pe, G - NS + 1)
                tt(xp[k][:, :], qv[:, 0, :], qv[:, 1, :], add).then_inc(sem_xp, 1)

        @block.tensor
        def _(tensor):
            tensor.wait_ge(sem_w, 16)
            for G in range(NG):
                k = G % NS
                tensor.wait_ge(sem_xp, G + 1)
                if G >= 2:  # psum bank reused; wait until ACT drained G-2
                    tensor.wait_ge(sem_actc, G - 1)
                ps = psum[G % 2]
                for lo, hi in ((0, 512), (512, 1024), (1024, 1152)):
                    mm = tensor.matmul(
                        ps[:, lo:hi], pwt[:, :], xp[k][:, lo:hi],
                        start=True, stop=True,
                    )
                mm.then_inc(sem_pe, 1)

        @block.scalar
        def _(scalar):
            for G in range(NG):
                scalar.wait_ge(sem_pe, G + 1)
                if G >= 4:  # res slot reused; wait until out-DMA of G-4 done
                    scalar.wait_ge(sem_out, 16 * (G - 3))
                r = res[G % 4]
                scalar.activation(
                    r[:, :], psum[G % 2][:, :],
                    mybir.ActivationFunctionType.Copy, scale=SCALE,
                ).then_inc(sem_actc, 1)
                scalar.wait_ge(sem_actc, G + 1)  # drain before own DMA reads it
                g = G % 8
                scalar.dma_start(out[32 * g : 32 * g + 32, :], r[:, :]).then_inc(
                    sem_out, 16
                )

    return nc


def _pool_weight():
    """One-hot lhsT [128, 32]: partition p=(oy2,dy,ox) -> column oy2*16+ox."""
    w = np.zeros((128, 32), np.float32)
    p = np.arange(128)
    w[p, (p // 64) * 16 + (p % 16)] = 1.0
    return w


def _grid_kidx(S_, O_):
    """kidx the reference computes for the regular 64x64 grid inputs."""
    xs = np.arange(S_, dtype=np.int64) % GRID
    ys = np.arange(S_, dtype=np.int64) // GRID
    return xs // KERN + (GRID // KERN) * (ys // KERN)


def _reference_numpy(hidden_states, kidx, padding_positions, O_):
    """Exact reference semantics on host (fallback for unstructured inputs)."""
    hs = np.where(padding_positions[..., None], 0.0, hidden_states).astype(np.float32)
    B_, S_, H_ = hs.shape
    k2 = S_ // O_
    out = np.zeros((B_, O_, H_), np.float32)
    counts = np.zeros((B_, O_), np.int64)
    for b in range(B_):
        np.add.at(out[b], kidx[b], hs[b])
        counts[b] = np.bincount(kidx[b], minlength=O_)[:O_]
    out = (out * np.float32(1.0 / k2)).astype(np.float32) * np.float32(
        float(H_) ** 0.5
    )
    return out, counts > 0


def kernel(hidden_states, pixel_position_ids, padding_positions, output_length):
    hs = np.asarray(hidden_states)
    pp = np.asarray(pixel_position_ids)
    pad = np.asarray(padding_positions)
    O_ = int(np.asarray(output_length))

    B_, S_, H_ = hs.shape
    kern = int((S_ // O_) ** 0.5)
    pos = np.maximum(pp.astype(np.int64), 0)
    max_x = pos[..., 0].max(axis=-1, keepdims=True) + 1
    kidx = pos[..., 0] // kern + (max_x // kern) * (pos[..., 1] // kern)

    counts = np.stack(
        [np.bincount(kidx[b], minlength=O_)[:O_] for b in range(B_)]
    )
    mask = counts > 0

    structured = (
        (B_, S_, H_, O_) == (B_EXPECTED, S, H, O)
        and (kidx == _grid_kidx(S_, O_)[None]).all()
    )
    if not structured:
        return _reference_numpy(hs, kidx, pad, O_)

    if pad.any():
        hs = np.where(pad[..., None], np.float32(0.0), hs)

    from concourse import bass_utils

    if "nc" not in _NC_CACHE:
        _NC_CACHE["nc"] = build_nc_v2(reps=1)
    nc = _NC_CACHE["nc"]

    pwv = _pool_weight()
    in_maps = [
        {"h": np.ascontiguousarray(hs[b], dtype=np.float32), "pw": pwv}
        for b in range(B_)
    ]
    r = bass_utils.run_bass_kernel_spmd(nc, in_maps, core_ids=list(range(N_CORES)))
    out = np.stack([r.results[c]["out"] for c in range(N_CORES)]).astype(hs.dtype)
    return out, mask


# revision 10
# speedup vs baseline: 1.0539x; 1.0539x over previous
"""Gemma4 vision pooler (segment-reduce 4x4 average pooling) on 8 Trainium2 cores.

Strategy: data-parallel over the batch axis — one image per NeuronCore.
The reference's segment ids (from the regular 64x64 patch-position grid)
reduce to a 4x4 average pool over the grid, so the per-segment gather is
expressed as strided DMA access patterns and the reduction as a DVE
pairwise-add tree.  Memory-bound: each core streams its 18.9 MB image in,
writes 1.2 MB out.

Active pipeline = build_nc_v2 (raw Bass, explicit semaphores; Tile is not
used because the staged walrus rejects multi-wait Drain instructions):
each of 8 input DMAs per image loads two grid rows (a fully contiguous
2.36 MB DRAM span, ~480 GB/s/core measured) onto partitions (oy%2, dy, ox);
DVE pools dx in the free dim, PE pools dy across partitions with a constant
one-hot fp32 matmul, ACT applies the sqrt(H)/16 scale draining PSUM and
stores 32 output rows per group on its own HWDGE ring.  build_nc (v1,
strided-gather + DVE-only tree, ~2x slower) is kept for reference.
Measured: ~43 us/image/core (~436 GB/s/core effective input BW).
"""

import numpy as np

S, H, O = 4096, 1152, 256
GRID, KERN = 64, 4
B_EXPECTED = 8
N_CORES = 8
SCALE = float(H) ** 0.5 / float(KERN * KERN)

_NC_CACHE: dict = {}


def build_nc(reps: int = 1):
    """Build the per-core Bass program. `reps` > 1 repeats the whole pipeline
    back-to-back in one NEFF (for on-device timing via wall-clock slope)."""
    from contextlib import ExitStack
    import concourse.bass as bass
    import concourse.mybir as mybir

    f32 = mybir.dt.float32
    nc = bass.Bass(trn_type="TRN2")
    h = nc.dram_tensor("h", [S, H], f32, kind="ExternalInput")
    out = nc.dram_tensor("out", [O, H], f32, kind="ExternalOutput")

    hap = h[:, :]
    # (oy, dy, ox, (dx e)): chunk source for (tile, dy) is v[t*8:(t+1)*8, dy]
    v = hap.rearrange("(oy dy ox dx) e -> oy dy ox (dx e)", oy=16, dy=4, ox=16, dx=4)
    # (oy, dy, ox, dx, e): slice source for (tile, j) is v2[t*8:(t+1)*8, 3, :, j]
    v2 = hap.rearrange("(oy dy ox dx) e -> oy dy ox dx e", oy=16, dy=4, ox=16, dx=4)

    with ExitStack() as ctx:
        sb = lambda name, free: ctx.enter_context(
            nc.sbuf_tensor(name, [128, free], f32)
        )
        c = [sb(f"c{dy}", 4608) for dy in range(3)]      # dy chunks, shared
        sl = [sb(f"sl{j}", 1152) for j in range(8)]      # dy=3 slices, per tile
        q0, q1 = sb("q0", 1152), sb("q1", 1152)
        y = [sb(f"y{dy}", 1152) for dy in range(3)]
        w, z = sb("w", 1152), sb("z", 1152)
        ra, rb = sb("ra", 1152), sb("rb", 1152)
        res = [sb(f"res{t}", 1152) for t in range(2)]

        # One semaphore per DMA-written buffer slot: completion order across
        # queued HWDGE DMAs is NOT guaranteed to be issue order (they fan out
        # over multiple queues), so "one shared sem >= 16*k" is unsound.
        sem_c = [ctx.enter_context(nc.semaphore(f"sem_c{dy}")) for dy in range(3)]
        sem_s = [ctx.enter_context(nc.semaphore(f"sem_s{j}")) for j in range(8)]
        sem_dve = ctx.enter_context(nc.semaphore("sem_dve"))
        sem_r = ctx.enter_context(nc.semaphore("sem_r"))
        sem_act = ctx.enter_context(nc.semaphore("sem_act"))
        sem_out = ctx.enter_context(nc.semaphore("sem_out"))

        # Semaphores retain values across NEFF executions on a core; a prior
        # run's leftovers make every wait_ge pass immediately (observed as
        # nondeterministic garbage).  Clear the whole kernel sem range, then
        # barrier via the NRT pseudo-barrier (its sems are outside the kernel
        # range, so it is safe while ours are still dirty).
        nc.gpsimd.dma_reset(nc._kernel_sem_range)
        nc.gpsimd.sem_clear(nc._kernel_sem_range)
        nc._nrt_pseudo_barrier()

        block = ctx.enter_context(nc.Block())

        @block.sync
        def _(sync):
            for i in range(reps):
                for t in range(2):
                    u = 2 * i + t  # global tile index
                    for dy in range(3):
                        if u > 0:  # chunk buffer reused; wait for y_dy of tile u-1
                            sync.wait_ge(sem_dve, 3 * (u - 1) + dy + 1)
                        sync.dma_start(
                            c[dy][:, :], v[t * 8 : (t + 1) * 8, dy]
                        ).then_inc(sem_c[dy], 16)
                    for j in range(4):
                        if i > 0:  # slice buffer reused; wait for tile (i-1, t) done
                            sync.wait_ge(sem_r, 2 * (i - 1) + t + 1)
                        sync.dma_start(
                            sl[4 * t + j][:, :], v2[t * 8 : (t + 1) * 8, 3, :, j]
                        ).then_inc(sem_s[4 * t + j], 16)
            sync.wait_ge(sem_out, 32 * reps)

        @block.vector
        def _(vector):
            add = mybir.AluOpType.add
            tt = vector.tensor_tensor
            for i in range(reps):
                for t in range(2):
                    u = 2 * i + t
                    for dy in range(3):
                        vector.wait_ge(sem_c[dy], 16 * (u + 1))
                        cc = c[dy][:, :].rearrange("p (dx e) -> p dx e", dx=4)
                        tt(q0[:, :], cc[:, 0], cc[:, 1], add)
                        tt(q1[:, :], cc[:, 2], cc[:, 3], add)
                        tt(y[dy][:, :], q0[:, :], q1[:, :], add).then_inc(sem_dve, 1)
                    tt(w[:, :], y[0][:, :], y[1][:, :], add)
                    tt(z[:, :], w[:, :], y[2][:, :], add)
                    vector.wait_ge(sem_s[4 * t + 0], 16 * (i + 1))
                    tt(ra[:, :], z[:, :], sl[4 * t + 0][:, :], add)
                    vector.wait_ge(sem_s[4 * t + 1], 16 * (i + 1))
                    if u > 0:  # rb still being read by ACT of tile u-1
                        vector.wait_ge(sem_act, u)
                    tt(rb[:, :], ra[:, :], sl[4 * t + 1][:, :], add)
                    vector.wait_ge(sem_s[4 * t + 2], 16 * (i + 1))
                    tt(ra[:, :], rb[:, :], sl[4 * t + 2][:, :], add)
                    vector.wait_ge(sem_s[4 * t + 3], 16 * (i + 1))
                    tt(rb[:, :], ra[:, :], sl[4 * t + 3][:, :], add).then_inc(sem_r, 1)

        @block.scalar
        def _(scalar):
            for i in range(reps):
                for t in range(2):
                    u = 2 * i + t
                    scalar.wait_ge(sem_r, u + 1)
                    if i > 0:  # res[t] still being read by out-DMA of (i-1, t)
                        scalar.wait_ge(sem_out, 32 * (i - 1) + 16 * (t + 1))
                    scalar.activation(
                        res[t][:, :],
                        rb[:, :],
                        mybir.ActivationFunctionType.Copy,
                        scale=SCALE,
                    ).then_inc(sem_act, 1)
                    # self-wait: ensure the activation's write has drained
                    # before this engine's DMA reads res[t]
                    scalar.wait_ge(sem_act, u + 1)
                    scalar.dma_start(
                        out[t * 128 : (t + 1) * 128, :], res[t][:, :]
                    ).then_inc(sem_out, 16)

    return nc


def build_nc_v2(reps: int = 1):
    """v2: partition = (oy%2, dy, ox), free = (dx, e).

    Each input DMA g in 0..7 loads grid rows oy=2g,2g+1 as ONE fully
    contiguous 2.36 MB DRAM span onto 128 partitions (measured 480 GB/s/core
    vs 194 for the v1 strided gather).  DVE pools dx in the free dim
    (2 adds/group), PE pools dy across partitions with a constant one-hot
    lhsT [128p -> 32 outputs] fp32 matmul (otherwise idle), ACT drains PSUM
    with the sqrt(H)/16 scale and stores 32 output rows per group.
    """
    from contextlib import ExitStack
    import concourse.bass as bass
    import concourse.mybir as mybir

    f32 = mybir.dt.float32
    nc = bass.Bass(trn_type="TRN2")
    h = nc.dram_tensor("h", [S, H], f32, kind="ExternalInput")
    pw = nc.dram_tensor("pw", [128, 32], f32, kind="ExternalInput")
    out = nc.dram_tensor("out", [O, H], f32, kind="ExternalOutput")

    # g-th slice: [oy2 2, dyox 64, (dx e) 4608] -> SBUF [128, 4608]
    vg = h[:, :].rearrange(
        "(g oy2 dyox dx) e -> g oy2 dyox (dx e)", g=8, oy2=2, dyox=64, dx=4
    )

    with ExitStack() as ctx:
        sb = lambda name, p, free: ctx.enter_context(
            nc.sbuf_tensor(name, [p, free], f32)
        )
        NS = 6                                               # in/xp ring depth
        tin = [sb(f"tin{k}", 128, 4608) for k in range(NS)]  # in slots
        q3 = sb("q3", 128, 2304)                             # dx pair sums
        xp = [sb(f"xp{k}", 128, 1152) for k in range(NS)]    # dx-pooled slots
        pwt = sb("pwt", 128, 32)                             # one-hot lhsT
        res = [sb(f"res{k}", 32, 1152) for k in range(4)]    # output staging
        psum = [
            ctx.enter_context(nc.psum_tensor(f"ps{k}", [32, 1152], f32))
            for k in range(2)
        ]

        sem_in = [ctx.enter_context(nc.semaphore(f"sem_in{k}")) for k in range(NS)]
        sem_w = ctx.enter_context(nc.semaphore("sem_w"))
        sem_a1 = ctx.enter_context(nc.semaphore("sem_a1"))
        sem_xp = ctx.enter_context(nc.semaphore("sem_xp"))
        sem_pe = ctx.enter_context(nc.semaphore("sem_pe"))
        sem_actc = ctx.enter_context(nc.semaphore("sem_actc"))
        sem_out = ctx.enter_context(nc.semaphore("sem_out"))

        nc.gpsimd.dma_reset(nc._kernel_sem_range)
        nc.gpsimd.sem_clear(nc._kernel_sem_range)
        nc._nrt_pseudo_barrier()

        block = ctx.enter_context(nc.Block())
        NG = 8 * reps  # total groups

        @block.sync
        def _(sync):
            sync.dma_start(pwt[:, :], pw[:, :]).then_inc(sem_w, 16)
            for G in range(NG):
                g = G % 8
                k = G % NS
                if G >= NS:  # slot reused; wait until add1 of G-NS consumed it
                    sync.wait_ge(sem_a1, G - NS + 1)
                sync.dma_start(tin[k][:, :], vg[g]).then_inc(sem_in[k], 16)
            sync.wait_ge(sem_out, 16 * NG)

        @block.vector
        def _(vector):
            add = mybir.AluOpType.add
            tt = vector.tensor_tensor
            for G in range(NG):
                k = G % NS
                vector.wait_ge(sem_in[k], 16 * (G // NS + 1))
                view = tin[k][:, :].rearrange("p (dx e) -> p dx e", dx=4)
                qv = q3[:, :].rearrange("p (j e) -> p j e", j=2)
                tt(qv, view[:, 0::2, :], view[:, 1::2, :], add).then_inc(sem_a1, 1)
                if G >= NS:  # xp slot reused; wait until PE of G-NS read it
                    vector.wait_ge(sem_pe, G - NS + 1)
                tt(xp[k][:, :], qv[:, 0, :], qv[:, 1, :], add).then_inc(sem_xp, 1)

        @block.tensor
        def _(tensor):
            tensor.wait_ge(sem_w, 16)
            for G in range(NG):
                k = G % NS
                tensor.wait_ge(sem_xp, G + 1)
                if G >= 2:  # psum bank reused; wait until ACT drained G-2
                    tensor.wait_ge(sem_actc, G - 1)
                ps = psum[G % 2]
                for lo, hi in ((0, 512), (512, 1024), (1024, 1152)):
                    mm = tensor.matmul(
                        ps[:, lo:hi], pwt[:, :], xp[k][:, lo:hi],
                        start=True, stop=True,
                    )
                mm.then_inc(sem_pe, 1)

        @block.scalar
        def _(scalar):
            for G in range(NG):
                scalar.wait_ge(sem_pe, G + 1)
                if G >= 4:  # res slot reused; wait until out-DMA of G-4 done
                    scalar.wait_ge(sem_out, 16 * (G - 3))
                r = res[G % 4]
                scalar.activation(
                    r[:, :], psum[G % 2][:, :],
                    mybir.ActivationFunctionType.Copy, scale=SCALE,
                ).then_inc(sem_actc, 1)
                scalar.wait_ge(sem_actc, G + 1)  # drain before own DMA reads it
                g = G % 8
                scalar.dma_start(out[32 * g : 32 * g + 32, :], r[:, :]).then_inc(
                    sem_out, 16
                )

    return nc


def _pool_weight():
    """One-hot lhsT [128, 32]: partition p=(oy2,dy,ox) -> column oy2*16+ox."""
    w = np.zeros((128, 32), np.float32)
    p = np.arange(128)
    w[p, (p // 64) * 16 + (p % 16)] = 1.0
    return w


def _grid_kidx(S_, O_):
    """kidx the reference computes for the regular 64x64 grid inputs."""
    xs = np.arange(S_, dtype=np.int64) % GRID
    ys = np.arange(S_, dtype=np.int64) // GRID
    return xs // KERN + (GRID // KERN) * (ys // KERN)


def _reference_numpy(hidden_states, kidx, padding_positions, O_):
    """Exact reference semantics on host (fallback for unstructured inputs)."""
    hs = np.where(padding_positions[..., None], 0.0, hidden_states).astype(np.float32)
    B_, S_, H_ = hs.shape
    k2 = S_ // O_
    out = np.zeros((B_, O_, H_), np.float32)
    counts = np.zeros((B_, O_), np.int64)
    for b in range(B_):
        np.add.at(out[b], kidx[b], hs[b])
        counts[b] = np.bincount(kidx[b], minlength=O_)[:O_]
    out = (out * np.float32(1.0 / k2)).astype(np.float32) * np.float32(
        float(H_) ** 0.5
    )
    return out, counts > 0


def kernel(hidden_states, pixel_position_ids, padding_positions, output_length):
    hs = np.asarray(hidden_states)
    pp = np.asarray(pixel_position_ids)
    pad = np.asarray(padding_positions)
    O_ = int(np.asarray(output_length))

    B_, S_, H_ = hs.shape
    kern = int((S_ // O_) ** 0.5)
    pos = np.maximum(pp.astype(np.int64), 0)
    max_x = pos[..., 0].max(axis=-1, keepdims=True) + 1
    kidx = pos[..., 0] // kern + (max_x // kern) * (pos[..., 1] // kern)

    counts = np.stack(
        [np.bincount(kidx[b], minlength=O_)[:O_] for b in range(B_)]
    )
    mask = counts > 0

    structured = (
        (B_, S_, H_, O_) == (B_EXPECTED, S, H, O)
        and (kidx == _grid_kidx(S_, O_)[None]).all()
    )
    if not structured:
        return _reference_numpy(hs, kidx, pad, O_)

    if pad.any():
        hs = np.where(pad[..., None], np.float32(0.0), hs)

    from concourse import bass_utils

    if "nc" not in _NC_CACHE:
        _NC_CACHE["nc"] = build_nc_v2(reps=1)
    nc = _NC_CACHE["nc"]

    pwv = _pool_weight()
    in_maps = [
        {"h": np.ascontiguousarray(hs[b], dtype=np.float32), "pw": pwv}
        for b in range(B_)
    ]
    r = bass_utils.run_bass_kernel_spmd(nc, in_maps, core_ids=list(range(N_CORES)))
    out = np.stack([r.results[c]["out"] for c in range(N_CORES)]).astype(hs.dtype)
    return out, mask


# revision 11
# speedup vs baseline: 1.3610x; 1.2914x over previous
"""Gemma4 vision pooler (segment-reduce 4x4 average pooling) on 8 Trainium2 cores.

Strategy: data-parallel over the batch axis — one image per NeuronCore.
The reference's segment ids (from the regular 64x64 patch-position grid)
reduce to a 4x4 average pool over the grid, so the per-segment gather is
expressed as strided DMA access patterns and the reduction as a DVE
pairwise-add tree.  Memory-bound: each core streams its 18.9 MB image in,
writes 1.2 MB out.

Active pipeline = build_nc_v2 (raw Bass, explicit semaphores; Tile is not
used because the staged walrus rejects multi-wait Drain instructions):
each of 8 input DMAs per image loads two grid rows (a fully contiguous
2.36 MB DRAM span, ~480 GB/s/core measured) onto partitions (oy%2, dy, ox);
DVE pools dx in the free dim, PE pools dy across partitions with a constant
one-hot fp32 matmul, ACT applies the sqrt(H)/16 scale draining PSUM and
stores 32 output rows per group on its own HWDGE ring.  build_nc (v1,
strided-gather + DVE-only tree, ~2x slower) is kept for reference.
Measured: ~43 us/image/core (~436 GB/s/core effective input BW).
"""

import numpy as np

S, H, O = 4096, 1152, 256
GRID, KERN = 64, 4
B_EXPECTED = 8
N_CORES = 8
SCALE = float(H) ** 0.5 / float(KERN * KERN)

_NC_CACHE: dict = {}


def build_nc(reps: int = 1):
    """Build the per-core Bass program. `reps` > 1 repeats the whole pipeline
    back-to-back in one NEFF (for on-device timing via wall-clock slope)."""
    from contextlib import ExitStack
    import concourse.bass as bass
    import concourse.mybir as mybir

    f32 = mybir.dt.float32
    nc = bass.Bass(trn_type="TRN2")
    h = nc.dram_tensor("h", [S, H], f32, kind="ExternalInput")
    out = nc.dram_tensor("out", [O, H], f32, kind="ExternalOutput")

    hap = h[:, :]
    # (oy, dy, ox, (dx e)): chunk source for (tile, dy) is v[t*8:(t+1)*8, dy]
    v = hap.rearrange("(oy dy ox dx) e -> oy dy ox (dx e)", oy=16, dy=4, ox=16, dx=4)
    # (oy, dy, ox, dx, e): slice source for (tile, j) is v2[t*8:(t+1)*8, 3, :, j]
    v2 = hap.rearrange("(oy dy ox dx) e -> oy dy ox dx e", oy=16, dy=4, ox=16, dx=4)

    with ExitStack() as ctx:
        sb = lambda name, free: ctx.enter_context(
            nc.sbuf_tensor(name, [128, free], f32)
        )
        c = [sb(f"c{dy}", 4608) for dy in range(3)]      # dy chunks, shared
        sl = [sb(f"sl{j}", 1152) for j in range(8)]      # dy=3 slices, per tile
        q0, q1 = sb("q0", 1152), sb("q1", 1152)
        y = [sb(f"y{dy}", 1152) for dy in range(3)]
        w, z = sb("w", 1152), sb("z", 1152)
        ra, rb = sb("ra", 1152), sb("rb", 1152)
        res = [sb(f"res{t}", 1152) for t in range(2)]

        # One semaphore per DMA-written buffer slot: completion order across
        # queued HWDGE DMAs is NOT guaranteed to be issue order (they fan out
        # over multiple queues), so "one shared sem >= 16*k" is unsound.
        sem_c = [ctx.enter_context(nc.semaphore(f"sem_c{dy}")) for dy in range(3)]
        sem_s = [ctx.enter_context(nc.semaphore(f"sem_s{j}")) for j in range(8)]
        sem_dve = ctx.enter_context(nc.semaphore("sem_dve"))
        sem_r = ctx.enter_context(nc.semaphore("sem_r"))
        sem_act = ctx.enter_context(nc.semaphore("sem_act"))
        sem_out = ctx.enter_context(nc.semaphore("sem_out"))

        # Semaphores retain values across NEFF executions on a core; a prior
        # run's leftovers make every wait_ge pass immediately (observed as
        # nondeterministic garbage).  Clear the whole kernel sem range, then
        # barrier via the NRT pseudo-barrier (its sems are outside the kernel
        # range, so it is safe while ours are still dirty).
        nc.gpsimd.dma_reset(nc._kernel_sem_range)
        nc.gpsimd.sem_clear(nc._kernel_sem_range)
        nc._nrt_pseudo_barrier()

        block = ctx.enter_context(nc.Block())

        @block.sync
        def _(sync):
            for i in range(reps):
                for t in range(2):
                    u = 2 * i + t  # global tile index
                    for dy in range(3):
                        if u > 0:  # chunk buffer reused; wait for y_dy of tile u-1
                            sync.wait_ge(sem_dve, 3 * (u - 1) + dy + 1)
                        sync.dma_start(
                            c[dy][:, :], v[t * 8 : (t + 1) * 8, dy]
                        ).then_inc(sem_c[dy], 16)
                    for j in range(4):
                        if i > 0:  # slice buffer reused; wait for tile (i-1, t) done
                            sync.wait_ge(sem_r, 2 * (i - 1) + t + 1)
                        sync.dma_start(
                            sl[4 * t + j][:, :], v2[t * 8 : (t + 1) * 8, 3, :, j]
                        ).then_inc(sem_s[4 * t + j], 16)
            sync.wait_ge(sem_out, 32 * reps)

        @block.vector
        def _(vector):
            add = mybir.AluOpType.add
            tt = vector.tensor_tensor
            for i in range(reps):
                for t in range(2):
                    u = 2 * i + t
                    for dy in range(3):
                        vector.wait_ge(sem_c[dy], 16 * (u + 1))
                        cc = c[dy][:, :].rearrange("p (dx e) -> p dx e", dx=4)
                        tt(q0[:, :], cc[:, 0], cc[:, 1], add)
                        tt(q1[:, :], cc[:, 2], cc[:, 3], add)
                        tt(y[dy][:, :], q0[:, :], q1[:, :], add).then_inc(sem_dve, 1)
                    tt(w[:, :], y[0][:, :], y[1][:, :], add)
                    tt(z[:, :], w[:, :], y[2][:, :], add)
                    vector.wait_ge(sem_s[4 * t + 0], 16 * (i + 1))
                    tt(ra[:, :], z[:, :], sl[4 * t + 0][:, :], add)
                    vector.wait_ge(sem_s[4 * t + 1], 16 * (i + 1))
                    if u > 0:  # rb still being read by ACT of tile u-1
                        vector.wait_ge(sem_act, u)
                    tt(rb[:, :], ra[:, :], sl[4 * t + 1][:, :], add)
                    vector.wait_ge(sem_s[4 * t + 2], 16 * (i + 1))
                    tt(ra[:, :], rb[:, :], sl[4 * t + 2][:, :], add)
                    vector.wait_ge(sem_s[4 * t + 3], 16 * (i + 1))
                    tt(rb[:, :], ra[:, :], sl[4 * t + 3][:, :], add).then_inc(sem_r, 1)

        @block.scalar
        def _(scalar):
            for i in range(reps):
                for t in range(2):
                    u = 2 * i + t
                    scalar.wait_ge(sem_r, u + 1)
                    if i > 0:  # res[t] still being read by out-DMA of (i-1, t)
                        scalar.wait_ge(sem_out, 32 * (i - 1) + 16 * (t + 1))
                    scalar.activation(
                        res[t][:, :],
                        rb[:, :],
                        mybir.ActivationFunctionType.Copy,
                        scale=SCALE,
                    ).then_inc(sem_act, 1)
                    # self-wait: ensure the activation's write has drained
                    # before this engine's DMA reads res[t]
                    scalar.wait_ge(sem_act, u + 1)
                    scalar.dma_start(
                        out[t * 128 : (t + 1) * 128, :], res[t][:, :]
                    ).then_inc(sem_out, 16)

    return nc


def build_nc_v2(reps: int = 1):
    """v2: partition = (oy%2, dy, ox), free = (dx, e).

    Each input DMA g in 0..7 loads grid rows oy=2g,2g+1 as ONE fully
    contiguous 2.36 MB DRAM span onto 128 partitions (measured 480 GB/s/core
    vs 194 for the v1 strided gather).  DVE pools dx in the free dim
    (2 adds/group), PE pools dy across partitions with a constant one-hot
    lhsT [128p -> 32 outputs] fp32 matmul (otherwise idle), ACT drains PSUM
    with the sqrt(H)/16 scale and stores 32 output rows per group.
    """
    from contextlib import ExitStack
    import concourse.bass as bass
    import concourse.mybir as mybir

    f32 = mybir.dt.float32
    nc = bass.Bass(trn_type="TRN2")
    h = nc.dram_tensor("h", [S, H], f32, kind="ExternalInput")
    pw = nc.dram_tensor("pw", [128, 32], f32, kind="ExternalInput")
    out = nc.dram_tensor("out", [O, H], f32, kind="ExternalOutput")

    # g-th slice: [oy2 2, dyox 64, (dx e) 4608] -> SBUF [128, 4608]
    vg = h[:, :].rearrange(
        "(g oy2 dyox dx) e -> g oy2 dyox (dx e)", g=8, oy2=2, dyox=64, dx=4
    )

    with ExitStack() as ctx:
        sb = lambda name, p, free: ctx.enter_context(
            nc.sbuf_tensor(name, [p, free], f32)
        )
        NS = 6                                               # in/xp ring depth
        tin = [sb(f"tin{k}", 128, 4608) for k in range(NS)]  # in slots
        q3 = sb("q3", 128, 2304)                             # dx pair sums
        xp = [sb(f"xp{k}", 128, 1152) for k in range(NS)]    # dx-pooled slots
        pwt = sb("pwt", 128, 32)                             # one-hot lhsT
        res = [sb(f"res{k}", 32, 1152) for k in range(4)]    # output staging
        psum = [
            ctx.enter_context(nc.psum_tensor(f"ps{k}", [32, 1152], f32))
            for k in range(2)
        ]

        sem_in = [ctx.enter_context(nc.semaphore(f"sem_in{k}")) for k in range(NS)]
        sem_w = ctx.enter_context(nc.semaphore("sem_w"))
        sem_a1 = ctx.enter_context(nc.semaphore("sem_a1"))
        sem_xp = ctx.enter_context(nc.semaphore("sem_xp"))
        sem_pe = ctx.enter_context(nc.semaphore("sem_pe"))
        sem_actc = ctx.enter_context(nc.semaphore("sem_actc"))
        sem_out = ctx.enter_context(nc.semaphore("sem_out"))

        nc.gpsimd.dma_reset(nc._kernel_sem_range)
        nc.gpsimd.sem_clear(nc._kernel_sem_range)
        nc._nrt_pseudo_barrier()

        block = ctx.enter_context(nc.Block())
        NG = 8 * reps  # total groups

        @block.sync
        def _(sync):
            sync.dma_start(pwt[:, :], pw[:, :]).then_inc(sem_w, 16)
            for G in range(NG):
                g = G % 8
                k = G % NS
                if G >= NS:  # slot reused; wait until add1 of G-NS consumed it
                    sync.wait_ge(sem_a1, G - NS + 1)
                sync.dma_start(tin[k][:, :], vg[g]).then_inc(sem_in[k], 16)
            sync.wait_ge(sem_out, 16 * NG)

        @block.vector
        def _(vector):
            add = mybir.AluOpType.add
            tt = vector.tensor_tensor
            for G in range(NG):
                k = G % NS
                vector.wait_ge(sem_in[k], 16 * (G // NS + 1))
                view = tin[k][:, :].rearrange("p (dx e) -> p dx e", dx=4)
                qv = q3[:, :].rearrange("p (j e) -> p j e", j=2)
                tt(qv, view[:, 0::2, :], view[:, 1::2, :], add).then_inc(sem_a1, 1)
                if G >= NS:  # xp slot reused; wait until PE of G-NS read it
                    vector.wait_ge(sem_pe, G - NS + 1)
                tt(xp[k][:, :], qv[:, 0, :], qv[:, 1, :], add).then_inc(sem_xp, 1)

        @block.tensor
        def _(tensor):
            tensor.wait_ge(sem_w, 16)
            for G in range(NG):
                k = G % NS
                tensor.wait_ge(sem_xp, G + 1)
                if G >= 2:  # psum bank reused; wait until ACT drained G-2
                    tensor.wait_ge(sem_actc, G - 1)
                ps = psum[G % 2]
                for lo, hi in ((0, 512), (512, 1024), (1024, 1152)):
                    mm = tensor.matmul(
                        ps[:, lo:hi], pwt[:, :], xp[k][:, lo:hi],
                        start=True, stop=True,
                    )
                mm.then_inc(sem_pe, 1)

        @block.scalar
        def _(scalar):
            for G in range(NG):
                scalar.wait_ge(sem_pe, G + 1)
                if G >= 4:  # res slot reused; wait until out-DMA of G-4 done
                    scalar.wait_ge(sem_out, 16 * (G - 3))
                r = res[G % 4]
                scalar.activation(
                    r[:, :], psum[G % 2][:, :],
                    mybir.ActivationFunctionType.Copy, scale=SCALE,
                ).then_inc(sem_actc, 1)
                scalar.wait_ge(sem_actc, G + 1)  # drain before own DMA reads it
                g = G % 8
                scalar.dma_start(out[32 * g : 32 * g + 32, :], r[:, :]).then_inc(
                    sem_out, 16
                )

    return nc


def build_nc_v3(reps: int = 1):
    """v3: partition = (oy%4, dy, ox//2), free = (8 x-values, e).

    One DMA = 4 grid rows = a single contiguous 4.72 MB span with 36 KB
    partition lines (the 645 GB/s profile; v2's 18 KB lines gave 480).
    Each partition holds TWO x-patches; DVE pools dx to xp[128, 2, 1152],
    PE scatters both j-halves into one [64, 1152] PSUM tile via two
    accumulating one-hot matmuls, ACT scales + stores 64 rows per group.
    """
    from contextlib import ExitStack
    import concourse.bass as bass
    import concourse.mybir as mybir

    f32 = mybir.dt.float32
    nc = bass.Bass(trn_type="TRN2")
    h = nc.dram_tensor("h", [S, H], f32, kind="ExternalInput")
    pw = nc.dram_tensor("pw", [128, 128], f32, kind="ExternalInput")
    out = nc.dram_tensor("out", [O, H], f32, kind="ExternalOutput")

    vg = h[:, :].rearrange(
        "(q oy4 dy ox2 dx8) e -> q (oy4 dy ox2) (dx8 e)",
        q=4, oy4=4, dy=4, ox2=8, dx8=8,
    )

    with ExitStack() as ctx:
        sb = lambda name, p, free: ctx.enter_context(
            nc.sbuf_tensor(name, [p, free], f32)
        )
        NS = 3
        tin = [sb(f"tin{k}", 128, 9216) for k in range(NS)]
        q4 = sb("q4", 128, 4608)
        xp = [sb(f"xp{k}", 128, 2304) for k in range(NS)]
        pwt = sb("pwt", 128, 128)
        res = [sb(f"res{k}", 64, 1152) for k in range(4)]
        psum = [
            ctx.enter_context(nc.psum_tensor(f"ps{k}", [64, 1152], f32))
            for k in range(2)
        ]

        sem_in = [ctx.enter_context(nc.semaphore(f"sem_in{k}")) for k in range(NS)]
        sem_w = ctx.enter_context(nc.semaphore("sem_w"))
        sem_a1 = ctx.enter_context(nc.semaphore("sem_a1"))
        sem_xp = ctx.enter_context(nc.semaphore("sem_xp"))
        sem_pe = ctx.enter_context(nc.semaphore("sem_pe"))
        sem_actc = ctx.enter_context(nc.semaphore("sem_actc"))
        sem_out = ctx.enter_context(nc.semaphore("sem_out"))

        nc.gpsimd.dma_reset(nc._kernel_sem_range)
        nc.gpsimd.sem_clear(nc._kernel_sem_range)
        nc._nrt_pseudo_barrier()

        block = ctx.enter_context(nc.Block())
        NG = 4 * reps

        @block.sync
        def _(sync):
            sync.dma_start(pwt[:, :], pw[:, :]).then_inc(sem_w, 16)
            for G in range(NG):
                q = G % 4
                k = G % NS
                if G >= NS:
                    sync.wait_ge(sem_a1, G - NS + 1)
                sync.dma_start(tin[k][:, :], vg[q]).then_inc(sem_in[k], 16)
            sync.wait_ge(sem_out, 16 * NG)

        @block.vector
        def _(vector):
            add = mybir.AluOpType.add
            tt = vector.tensor_tensor
            for G in range(NG):
                k = G % NS
                vector.wait_ge(sem_in[k], 16 * (G // NS + 1))
                view = tin[k][:, :].rearrange("p (dx e) -> p dx e", dx=8)
                q4v = q4[:, :].rearrange("p (m e) -> p m e", m=4)
                tt(q4v, view[:, 0::2, :], view[:, 1::2, :], add).then_inc(sem_a1, 1)
                if G >= NS:
                    vector.wait_ge(sem_pe, G - NS + 1)
                xpv = xp[k][:, :].rearrange("p (j e) -> p j e", j=2)
                tt(xpv, q4v[:, 0::2, :], q4v[:, 1::2, :], add).then_inc(sem_xp, 1)

        @block.tensor
        def _(tensor):
            tensor.wait_ge(sem_w, 16)
            for G in range(NG):
                k = G % NS
                tensor.wait_ge(sem_xp, G + 1)
                if G >= 2:
                    tensor.wait_ge(sem_actc, G - 1)
                ps = psum[G % 2]
                for lo, hi in ((0, 512), (512, 1024), (1024, 1152)):
                    for j in range(2):
                        mm = tensor.matmul(
                            ps[:, lo:hi],
                            pwt[:, 64 * j : 64 * j + 64],
                            xp[k][:, 1152 * j + lo : 1152 * j + hi],
                            start=(j == 0), stop=(j == 1),
                        )
                mm.then_inc(sem_pe, 1)

        @block.scalar
        def _(scalar):
            for G in range(NG):
                scalar.wait_ge(sem_pe, G + 1)
                if G >= 4:
                    scalar.wait_ge(sem_out, 16 * (G - 3))
                r = res[G % 4]
                scalar.activation(
                    r[:, :], psum[G % 2][:, :],
                    mybir.ActivationFunctionType.Copy, scale=SCALE,
                ).then_inc(sem_actc, 1)
                scalar.wait_ge(sem_actc, G + 1)
                q = G % 4
                scalar.dma_start(out[64 * q : 64 * q + 64, :], r[:, :]).then_inc(
                    sem_out, 16
                )

    return nc


def _pool_weight_v3():
    """Two one-hot lhsT [128, 64] packed as [128, 128]:
    partition p=(oy4,dy,ox2) -> column oy4*16 + 2*ox2 + j (j = x-patch half)."""
    w = np.zeros((128, 128), np.float32)
    p = np.arange(128)
    for j in range(2):
        w[p, 64 * j + (p // 32) * 16 + 2 * (p % 8) + j] = 1.0
    return w


def _pool_weight():
    """One-hot lhsT [128, 32]: partition p=(oy2,dy,ox) -> column oy2*16+ox."""
    w = np.zeros((128, 32), np.float32)
    p = np.arange(128)
    w[p, (p // 64) * 16 + (p % 16)] = 1.0
    return w


def _grid_kidx(S_, O_):
    """kidx the reference computes for the regular 64x64 grid inputs."""
    xs = np.arange(S_, dtype=np.int64) % GRID
    ys = np.arange(S_, dtype=np.int64) // GRID
    return xs // KERN + (GRID // KERN) * (ys // KERN)


def _reference_numpy(hidden_states, kidx, padding_positions, O_):
    """Exact reference semantics on host (fallback for unstructured inputs)."""
    hs = np.where(padding_positions[..., None], 0.0, hidden_states).astype(np.float32)
    B_, S_, H_ = hs.shape
    k2 = S_ // O_
    out = np.zeros((B_, O_, H_), np.float32)
    counts = np.zeros((B_, O_), np.int64)
    for b in range(B_):
        np.add.at(out[b], kidx[b], hs[b])
        counts[b] = np.bincount(kidx[b], minlength=O_)[:O_]
    out = (out * np.float32(1.0 / k2)).astype(np.float32) * np.float32(
        float(H_) ** 0.5
    )
    return out, counts > 0


def kernel(hidden_states, pixel_position_ids, padding_positions, output_length):
    hs = np.asarray(hidden_states)
    pp = np.asarray(pixel_position_ids)
    pad = np.asarray(padding_positions)
    O_ = int(np.asarray(output_length))

    B_, S_, H_ = hs.shape
    kern = int((S_ // O_) ** 0.5)
    pos = np.maximum(pp.astype(np.int64), 0)
    max_x = pos[..., 0].max(axis=-1, keepdims=True) + 1
    kidx = pos[..., 0] // kern + (max_x // kern) * (pos[..., 1] // kern)

    counts = np.stack(
        [np.bincount(kidx[b], minlength=O_)[:O_] for b in range(B_)]
    )
    mask = counts > 0

    structured = (
        (B_, S_, H_, O_) == (B_EXPECTED, S, H, O)
        and (kidx == _grid_kidx(S_, O_)[None]).all()
    )
    if not structured:
        return _reference_numpy(hs, kidx, pad, O_)

    if pad.any():
        hs = np.where(pad[..., None], np.float32(0.0), hs)

    from concourse import bass_utils

    if "nc" not in _NC_CACHE:
        _NC_CACHE["nc"] = build_nc_v2(reps=1)
    nc = _NC_CACHE["nc"]

    pwv = _pool_weight()
    in_maps = [
        {"h": np.ascontiguousarray(hs[b], dtype=np.float32), "pw": pwv}
        for b in range(B_)
    ]
    r = bass_utils.run_bass_kernel_spmd(nc, in_maps, core_ids=list(range(N_CORES)))
    out = np.stack([r.results[c]["out"] for c in range(N_CORES)]).astype(hs.dtype)
    return out, mask
